# revision 3
# baseline (speedup 1.0000x reference)
"""Multi-head attention (B=4, N=2048, D=1024, H=16) on 8 Trainium2 cores.

Sharding: core = (batch b, head-group hg) -> 4 batches x 2 groups of 8 heads.
Each core computes, for its batch and its 8 heads:
    QK^T projections (transposed layout), V (natural layout),
    scores S^T = K^T.T-free einsum via PE, exp via ACT, P~V + denominators
    via col-packed PE matmuls, deferred softmax normalization, and a partial
    output projection against its 512 rows of W_proj.
Host sums the two per-batch partials and adds b_proj.

All matmuls run in fp32r (full-rate reduced-precision fp32) except PV/denoms
which run in bf16 (P~ produced by ACT exp in bf16); softmax numerator and
denominator share the same bf16 P~, so the rounding largely cancels.

No attention max-subtraction is needed: scores are ~N(0, 0.17) so exp() is
comfortably in range (softmax is shift-invariant; reference equality holds
to fp32 rounding).
"""

import sys

if "/opt/trn_rl_repo" not in sys.path:
    sys.path.insert(0, "/opt/trn_rl_repo")

from contextlib import ExitStack

import numpy as np

B, N, D, H = 4, 2048, 1024, 16
HG = 2                 # head groups (tensor parallel)
NCORES = B * HG        # 8
DH = D // HG           # 512 features per group = 8 heads * 64
P = 128
KC = D // P            # 8 contraction chunks over d_model
MC = 2 * DH // P       # 8 feature chunks of [Q|K]
NT = N // 512          # 4 token 512-chunks
TJ = N // P            # 16 token 128-chunks (the attention j axis)
IB = 1024              # i-block (exp free-dim)
NI = N // IB           # 2
IQ = IB // 512         # 2 matmul free-dim quarters per i-block
CP = 4                 # head pairs per core
SCALE = (D // H) ** -0.5

_cached = {}


def _build():
    import concourse.mybir as mybir
    import concourse.tile as tile
    from concourse import bacc

    f32 = mybir.dt.float32
    f32r = mybir.dt.float32r
    bf16 = mybir.dt.bfloat16
    AF = mybir.ActivationFunctionType

    nc = bacc.Bacc("TRN2", target_bir_lowering=False, debug=False,
                   enable_asserts=False)

    xt = nc.dram_tensor("xt", (D, N), f32r, kind="ExternalInput").ap()
    wqk = nc.dram_tensor("wqk", (D, 2 * DH), f32r, kind="ExternalInput").ap()
    wv = nc.dram_tensor("wv", (D, DH), f32r, kind="ExternalInput").ap()
    wp = nc.dram_tensor("wp", (DH, D), f32r, kind="ExternalInput").ap()
    bqk = nc.dram_tensor("bqk", (1, 2 * DH), f32r, kind="ExternalInput").ap()
    bv = nc.dram_tensor("bv", (1, DH), f32r, kind="ExternalInput").ap()
    y = nc.dram_tensor("y", (N, D), f32, kind="ExternalOutput").ap()

    with tile.TileContext(nc) as tc, ExitStack() as ctx:
        const = ctx.enter_context(tc.tile_pool(name="const", bufs=1))
        persist = ctx.enter_context(tc.tile_pool(name="persist", bufs=1))

        ones_f32 = const.tile([1, 512], f32)
        nc.vector.memset(ones_f32[:], 1.0)
        ones_row = const.tile([1, 512], f32r)
        nc.vector.tensor_copy(ones_row[:], ones_f32[:])
        ones_col = const.tile([P, 1], bf16)
        nc.vector.memset(ones_col[:], 1.0)
        bqk_sb = const.tile([1, 2 * DH], f32r)
        nc.sync.dma_start(bqk_sb[:], bqk)
        bv_sb = const.tile([1, DH], f32r)
        nc.sync.dma_start(bv_sb[:], bv)

        qt = persist.tile([P, MC // 2, N], f32r)      # Q^T  [128, 4, 2048]
        kt = persist.tile([P, MC // 2, N], f32r)      # K^T  [128, 4, 2048]
        vsb = persist.tile([P, TJ, DH], bf16)         # V    [128, 16, 512]

        # ---------------- Phase 1: QKV projections ----------------
        with ExitStack() as c1:
            wpool = c1.enter_context(tc.tile_pool(name="w1", bufs=1))
            xpool = c1.enter_context(tc.tile_pool(name="xt", bufs=2))
            ps1 = c1.enter_context(tc.tile_pool(name="ps1", bufs=4, space="PSUM"))

            wqk_sb = wpool.tile([P, KC, 2 * DH], f32r)
            nc.sync.dma_start(wqk_sb[:], wqk.rearrange("(ko p) m -> p ko m", p=P))
            wv_sb = wpool.tile([P, KC, DH], f32r)
            nc.sync.dma_start(wv_sb[:], wv.rearrange("(ko p) m -> p ko m", p=P))

            xt_r = xt.rearrange("(ko p) t -> p ko t", p=P)
            for n in range(NT):
                xt_t = xpool.tile([P, KC, 512], f32r, tag="xt")
                nc.sync.dma_start(xt_t[:], xt_r[:, :, n * 512:(n + 1) * 512])
                # QK^T: out [feat 128, tok 512]
                for m in range(MC):
                    pt = ps1.tile([P, 512], f32, tag="ps_qk")
                    for k in range(KC):
                        nc.tensor.matmul(pt[:], wqk_sb[:, k, m * P:(m + 1) * P],
                                         xt_t[:, k, :], start=(k == 0), stop=False)
                    nc.tensor.matmul(pt[:], bqk_sb[0:1, m * P:(m + 1) * P],
                                     ones_row[:], start=False, stop=True)
                    dst = qt if m < MC // 2 else kt
                    nc.vector.tensor_copy(dst[:, m % (MC // 2), n * 512:(n + 1) * 512],
                                          pt[:])
                # V natural: out [tok 128, vfeat 512]
                for tt in range(4):
                    t = n * 4 + tt
                    pv = ps1.tile([P, DH], f32, tag="ps_v")
                    for k in range(KC):
                        nc.tensor.matmul(pv[:], xt_t[:, k, tt * P:(tt + 1) * P],
                                         wv_sb[:, k, :], start=(k == 0), stop=False)
                    nc.tensor.matmul(pv[:], ones_row[0:1, 0:P], bv_sb[:],
                                     start=False, stop=True)
                    nc.vector.tensor_copy(vsb[:, t, :], pv[:])

        # ---------------- Phase 2: attention + projection ----------------
        with ExitStack() as c2:
            p2 = c2.enter_context(tc.tile_pool(name="p2", bufs=1))
            ppool = c2.enter_context(tc.tile_pool(name="pp", bufs=3))
            otpool = c2.enter_context(tc.tile_pool(name="ot", bufs=2))
            dpool = c2.enter_context(tc.tile_pool(name="dv", bufs=2))
            ypool = c2.enter_context(tc.tile_pool(name="yb", bufs=3))
            ps_s = c2.enter_context(tc.tile_pool(name="ps_s", bufs=2, space="PSUM"))
            ps_o = c2.enter_context(tc.tile_pool(name="ps_o", bufs=1, space="PSUM"))

            wp_sb = p2.tile([P, DH // P, D], f32r)    # [128, 4, 1024]
            nc.sync.dma_start(wp_sb[:], wp.rearrange("(c p) o -> p c o", p=P))

            for i in range(NI):
                ot_i = otpool.tile([P, CP, IB], f32r, tag="ot")
                for c in range(CP):
                    oab = ps_o.tile([P, IB], f32, tag="oab")
                    dden = ps_o.tile([33, IB], f32, tag="den")
                    for j in range(TJ):
                        s_a = ps_s.tile([P, IB], f32, tag="s")
                        s_b = ps_s.tile([P, IB], f32, tag="s")
                        ksl = slice(j * P, (j + 1) * P)
                        for iq in range(IQ):
                            isl = slice(i * IB + iq * 512, i * IB + (iq + 1) * 512)
                            osl = slice(iq * 512, (iq + 1) * 512)
                            # row-packed score matmuls: head A rows 0:64,
                            # head B rows 64:128 of the qk feature chunk c
                            nc.tensor.matmul(s_a[:, osl], kt[0:64, c, ksl],
                                             qt[0:64, c, isl], start=True, stop=True)
                            nc.tensor.matmul(s_b[:, osl], kt[64:128, c, ksl],
                                             qt[64:128, c, isl], start=True, stop=True)
                        p_a = ppool.tile([P, IB], bf16, tag="pa")
                        nc.scalar.activation(p_a[:], s_a[:], AF.Exp, scale=SCALE)
                        p_b = ppool.tile([P, IB], bf16, tag="pb")
                        nc.scalar.activation(p_b[:], s_b[:], AF.Exp, scale=SCALE)
                        st = (j == 0)
                        sp = (j == TJ - 1)
                        for iq in range(IQ):
                            osl = slice(iq * 512, (iq + 1) * 512)
                            # col-packed PV: head A -> out rows 0:64,
                            # head B -> out rows 64:128
                            nc.tensor.matmul(oab[0:64, osl],
                                             vsb[:, j, c * P:c * P + 64],
                                             p_a[:, osl], start=st, stop=sp)
                            nc.tensor.matmul(oab[64:128, osl],
                                             vsb[:, j, c * P + 64:(c + 1) * P],
                                             p_b[:, osl], start=st, stop=sp)
                            # denominators: rows 0 and 32 (col groups 0 / 1)
                            nc.tensor.matmul(dden[0:1, osl], ones_col[:],
                                             p_a[:, osl], start=st, stop=sp)
                            nc.tensor.matmul(dden[32:33, osl], ones_col[:],
                                             p_b[:, osl], start=st, stop=sp)
                    # softmax normalization, deferred: O / denom
                    dra = dpool.tile([1, IB], f32, tag="dra")
                    nc.vector.reciprocal(dra[:], dden[0:1, :])
                    drb = dpool.tile([1, IB], f32, tag="drb")
                    nc.vector.reciprocal(drb[:], dden[32:33, :])
                    dba = dpool.tile([P, IB], f32, tag="dba")
                    nc.gpsimd.partition_broadcast(dba[0:64, :], dra[:])
                    dbb = dpool.tile([P, IB], f32, tag="dbb")
                    nc.gpsimd.partition_broadcast(dbb[:], drb[:])
                    nc.vector.tensor_mul(ot_i[0:64, c, :], oab[0:64, :],
                                         dba[0:64, :])
                    nc.vector.tensor_mul(ot_i[64:128, c, :], oab[64:128, :],
                                         dbb[64:128, :])
                # output projection for this i-block
                for t in range(IB // P):
                    for o in range(D // 512):
                        yp_full = ps_s.tile([P, IB], f32, tag="s", name="yp")
                        yp = yp_full[:, 0:512]
                        for cc in range(CP):
                            nc.tensor.matmul(yp[:],
                                             ot_i[:, cc, t * P:(t + 1) * P],
                                             wp_sb[:, cc, o * 512:(o + 1) * 512],
                                             start=(cc == 0), stop=(cc == CP - 1))
                        ysb = ypool.tile([P, 512], f32, tag="y")
                        nc.vector.tensor_copy(ysb[:], yp[:])
                        r0 = i * IB + t * P
                        nc.sync.dma_start(y[r0:r0 + P, o * 512:(o + 1) * 512],
                                          ysb[:])

    nc.compile()
    return nc


def _get_nc():
    if "nc" not in _cached:
        _cached["nc"] = _build()
    return _cached["nc"]


def kernel(x, W_qkv, b_qkv, W_proj, b_proj):
    from concourse.bass_utils import run_bass_kernel_spmd

    x = np.asarray(x, dtype=np.float32)
    W_qkv = np.asarray(W_qkv, dtype=np.float32)
    b_qkv = np.asarray(b_qkv, dtype=np.float32)
    W_proj = np.asarray(W_proj, dtype=np.float32)
    b_proj = np.asarray(b_proj, dtype=np.float32)

    in_maps = []
    for core in range(NCORES):
        b, hg = divmod(core, HG)
        hs = slice(DH * hg, DH * (hg + 1))
        in_maps.append({
            "xt": np.ascontiguousarray(x[b].T),
            "wqk": np.ascontiguousarray(
                np.concatenate([W_qkv[:, hs],
                                W_qkv[:, D + DH * hg:D + DH * (hg + 1)]], axis=1)),
            "wv": np.ascontiguousarray(W_qkv[:, 2 * D + DH * hg:2 * D + DH * (hg + 1)]),
            "wp": np.ascontiguousarray(W_proj[DH * hg:DH * (hg + 1), :]),
            "bqk": np.concatenate([b_qkv[hs],
                                   b_qkv[D + DH * hg:D + DH * (hg + 1)]])[None, :],
            "bv": b_qkv[2 * D + DH * hg:2 * D + DH * (hg + 1)][None, :],
        })

    nc = _get_nc()
    res = run_bass_kernel_spmd(nc, in_maps, core_ids=list(range(NCORES)))
    out = np.empty((B, N, D), dtype=np.float32)
    for b in range(B):
        out[b] = res.results[2 * b]["y"] + res.results[2 * b + 1]["y"] + b_proj
    return out
